# revision 2
# baseline (speedup 1.0000x reference)
"""GQA attention block on 8 trn2 NeuronCores.

Sharding: core c = (batch b=c//4, kv-head-pair g=c%4). Each core owns kv heads
{2g, 2g+1} and their 8 query heads (GQA tile mapping: q-head i -> kv-head i%8),
with Wq/Wk/Wv column-sharded and Wo row-sharded; host sums the 4 partial
outputs per batch and adds bo.

Device layout strategy (per core):
  - host stages q^T/k^T/v^T (bf16) so every matmul has its contraction dim on
    partitions with no device-side transposes.
  - RoPE folded into doubled projection weights Wt=[W | rot(W)] (host permuted)
    + elementwise cos/sin combine on DVE.
  - attention computed transposed: scores^T[k,q] = Kp^T(chunk)ᵀ·Qp^T, exp on
    ACT (scale=1/8 folded in, no max subtraction -- scores bounded ~|6|),
    AV via lhsT=Vp with an appended ones column giving the softmax denominator
    for free; normalization via reciprocal + K=1 outer-product broadcast.
  - out^T feeds the final projection as lhsT directly; partial [S,D] fp32 out.
"""

import os
from contextlib import ExitStack

import numpy as np
import ml_dtypes

D = 2048
QH = 32
KVH = 8
HD = 64
B = 2
S = 2048
THETA = 1000000.0
P = 128
NCORES = 8

BF16 = ml_dtypes.bfloat16

_CACHE = {}


def _build_program():
    import concourse.bass as bass
    import concourse.tile as tile
    from concourse import bacc, mybir

    nc = bacc.Bacc(
        "TRN2",
        target_bir_lowering=False,
        debug=False,
        enable_asserts=False,
        num_devices=NCORES,
    )
    bf = mybir.dt.bfloat16
    f32 = mybir.dt.float32

    qT = nc.dram_tensor("qT", [D, S], bf, kind="ExternalInput").ap()
    kT = nc.dram_tensor("kT", [D, S], bf, kind="ExternalInput").ap()
    vT = nc.dram_tensor("vT", [D, S], bf, kind="ExternalInput").ap()
    wqt = nc.dram_tensor("wqt", [D, 1024], bf, kind="ExternalInput").ap()
    wkt = nc.dram_tensor("wkt", [D, 256], bf, kind="ExternalInput").ap()
    wv = nc.dram_tensor("wv", [D, 128], bf, kind="ExternalInput").ap()
    wo = nc.dram_tensor("wo", [512, D], bf, kind="ExternalInput").ap()
    cosr = nc.dram_tensor("cosr", [P, S], f32, kind="ExternalInput").ap()
    sinr = nc.dram_tensor("sinr", [P, S], f32, kind="ExternalInput").ap()
    out = nc.dram_tensor("out", [S, D], f32, kind="ExternalOutput").ap()

    # partitioned DRAM views
    qT3 = qT.rearrange("(o p) s -> p o s", p=P)    # [128, 16, 2048]
    kT3 = kT.rearrange("(o p) s -> p o s", p=P)
    vT3 = vT.rearrange("(o p) s -> p o s", p=P)
    wqt3 = wqt.rearrange("(o p) m -> p o m", p=P)  # [128, 16, 1024]
    wkt3 = wkt.rearrange("(o p) m -> p o m", p=P)  # [128, 16, 256]
    wv3 = wv.rearrange("(o p) m -> p o m", p=P)    # [128, 16, 128]
    wo3 = wo.rearrange("(o p) d -> p o d", p=P)    # [128, 4, 2048]
    out3 = out.rearrange("(t p) d -> p t d", p=P)  # [128, 16, 2048]

    with tile.TileContext(nc) as tc, ExitStack() as ctx:
        const = ctx.enter_context(tc.tile_pool(name="const", bufs=1))
        persist = ctx.enter_context(tc.tile_pool(name="persist", bufs=1))

        # ---- resident weights / tables ----
        wqt_sb = const.tile([P, 16, 1024], bf, tag="wqt")
        nc.sync.dma_start(wqt_sb[:], wqt3[:])
        wkt_sb = const.tile([P, 16, 256], bf, tag="wkt")
        nc.sync.dma_start(wkt_sb[:], wkt3[:])
        wv_sb = const.tile([P, 16, 128], bf, tag="wv")
        nc.sync.dma_start(wv_sb[:], wv3[:])
        wo_sb = const.tile([P, 4, 2048], bf, tag="wo")
        nc.sync.dma_start(wo_sb[:], wo3[:])
        cos_sb = const.tile([P, S], f32, tag="cos")
        nc.sync.dma_start(cos_sb[:], cosr[:])
        sin_sb = const.tile([P, S], f32, tag="sin")
        nc.sync.dma_start(sin_sb[:], sinr[:])
        ones_sb = const.tile([1, 64], f32, tag="ones")
        nc.vector.memset(ones_sb[:], 1.0)

        # ---- persistent intermediates ----
        kpt_b = persist.tile([64, 2, S], bf, tag="kpt")      # rotated K^T per kv head
        qpt_b = persist.tile([64, 8, 2048], bf, tag="qpt")   # rotated Q^T per q head
        vp_sb = persist.tile([P, 16, 130], bf, tag="vp")     # Vp + ones cols
        outT_b = persist.tile([P, 4, 2048], bf, tag="outT")  # unnormalized out^T
        nc.vector.memset(vp_sb[:, :, 64:65], 1.0)
        nc.vector.memset(vp_sb[:, :, 129:130], 1.0)

        # =============== Phase 1-3: projections ===============
        with ExitStack() as pctx:
            bigin = pctx.enter_context(tc.tile_pool(name="bigin", bufs=1))
            kstream = pctx.enter_context(tc.tile_pool(name="kstream", bufs=4))
            ptmp = pctx.enter_context(tc.tile_pool(name="ptmp", bufs=2))
            ppsum = pctx.enter_context(
                tc.tile_pool(name="ppsum", bufs=4, space="PSUM")
            )

            # ---- V projection: direct Vp [s,128] via lhsT = vT slices ----
            for quarter in range(4):
                vh_sb = bigin.tile([P, 16, 512], bf, tag="bigin")
                for o in range(16):
                    nc.sync.dma_start(
                        vh_sb[:, o, :], vT3[:, o, quarter * 512 : (quarter + 1) * 512]
                    )
                for st in range(4):  # s-tiles of 128 within this quarter
                    psv_full = ppsum.tile([P, 512], f32, tag="pp", name="psv")
                    psv = psv_full[:, :128]
                    for o in range(16):
                        nc.tensor.matmul(
                            psv,
                            lhsT=vh_sb[:, o, st * 128 : (st + 1) * 128],
                            rhs=wv_sb[:, o, :],
                            start=(o == 0),
                            stop=(o == 15),
                        )
                    kt_idx = quarter * 4 + st
                    nc.vector.tensor_copy(out=vp_sb[:, kt_idx, 0:64], in_=psv[:, 0:64])
                    nc.vector.tensor_copy(
                        out=vp_sb[:, kt_idx, 65:129], in_=psv[:, 64:128]
                    )

            # ---- K projection + RoPE: KpT_rot per kv head ----
            for ns in range(4):
                ps_kp = ppsum.tile([P, 512], f32, tag="pp")
                ps_kr = ppsum.tile([P, 512], f32, tag="pp")
                for o in range(16):
                    ktile = kstream.tile([P, 512], bf, tag="kt")
                    nc.sync.dma_start(
                        ktile[:], kT3[:, o, ns * 512 : (ns + 1) * 512]
                    )
                    nc.tensor.matmul(
                        ps_kp,
                        lhsT=wkt_sb[:, o, 0:128],
                        rhs=ktile[:],
                        start=(o == 0),
                        stop=(o == 15),
                    )
                    nc.tensor.matmul(
                        ps_kr,
                        lhsT=wkt_sb[:, o, 128:256],
                        rhs=ktile[:],
                        start=(o == 0),
                        stop=(o == 15),
                    )
                sl = slice(ns * 512, (ns + 1) * 512)
                t1 = ptmp.tile([P, 512], f32, tag="t1")
                t2 = ptmp.tile([P, 512], f32, tag="t2")
                nc.vector.tensor_mul(out=t1[:], in0=ps_kp[:], in1=cos_sb[:, sl])
                nc.vector.tensor_mul(out=t2[:], in0=ps_kr[:], in1=sin_sb[:, sl])
                for lh in range(2):
                    lp = slice(lh * 64, lh * 64 + 64)
                    nc.vector.tensor_add(
                        out=kpt_b[:, lh, sl], in0=t1[lp, :], in1=t2[lp, :]
                    )

        # ======= unified pipeline: per s-quarter Qproj -> attn -> outproj =======
        with ExitStack() as mctx:
            bigin = mctx.enter_context(tc.tile_pool(name="bigin2", bufs=1))
            ptmp = mctx.enter_context(tc.tile_pool(name="ptmp2", bufs=2))
            mpsum = mctx.enter_context(
                tc.tile_pool(name="mpsum", bufs=3, space="PSUM")
            )
            apsum = mctx.enter_context(
                tc.tile_pool(name="apsum", bufs=3, space="PSUM")
            )
            opsum = mctx.enter_context(
                tc.tile_pool(name="opsum", bufs=2, space="PSUM")
            )
            epool = mctx.enter_context(tc.tile_pool(name="et", bufs=24))
            ntmp = mctx.enter_context(tc.tile_pool(name="ntmp", bufs=3))
            fout = mctx.enter_context(tc.tile_pool(name="fout", bufs=3))
            scale = 1.0 / float(np.sqrt(HD))
            Exp = mybir.ActivationFunctionType.Exp

            for quarter in range(4):
                # ---- Q projection + RoPE for this s-quarter ----
                qh_sb = bigin.tile([P, 16, 512], bf, tag="bigin")
                for o in range(16):
                    nc.sync.dma_start(
                        qh_sb[:, o, :], qT3[:, o, quarter * 512 : (quarter + 1) * 512]
                    )
                for m in range(4):
                    ps_qp = mpsum.tile([P, 512], f32, tag="pp")
                    for o in range(16):
                        nc.tensor.matmul(
                            ps_qp,
                            lhsT=wqt_sb[:, o, m * 128 : (m + 1) * 128],
                            rhs=qh_sb[:, o, :],
                            start=(o == 0),
                            stop=(o == 15),
                        )
                    gs = slice(quarter * 512, (quarter + 1) * 512)
                    # rotate_half via 32-aligned partition-shifted DVE copies
                    rot = ptmp.tile([P, 512], f32, tag="rot")
                    for hh in range(2):
                        b0 = hh * 64
                        nc.vector.tensor_scalar_mul(
                            rot[b0 : b0 + 32, :], ps_qp[b0 + 32 : b0 + 64, :], -1.0
                        )
                        nc.vector.tensor_copy(
                            out=rot[b0 + 32 : b0 + 64, :], in_=ps_qp[b0 : b0 + 32, :]
                        )
                    t1 = ptmp.tile([P, 512], f32, tag="t1")
                    t2 = ptmp.tile([P, 512], f32, tag="t2")
                    nc.vector.tensor_mul(out=t1[:], in0=ps_qp[:], in1=cos_sb[:, gs])
                    nc.vector.tensor_mul(out=t2[:], in0=rot[:], in1=sin_sb[:, gs])
                    for sub in range(2):
                        lp = slice(sub * 64, sub * 64 + 64)
                        nc.vector.tensor_add(
                            out=qpt_b[:, 2 * m + sub, gs],
                            in0=t1[lp, :],
                            in1=t2[lp, :],
                        )

                # ---- attention for sc = quarter ----
                for lh in range(2):
                    for j in range(4):
                        h = lh * 4 + j
                        hp = slice((h % 2) * 64, (h % 2) * 64 + 64)
                        hc = h // 2
                        ssl = slice(quarter * 512, (quarter + 1) * 512)
                        pso = opsum.tile([65, 512], f32, tag="po")
                        for kt in range(16):
                            pss = apsum.tile([P, 512], f32, tag="ps")
                            nc.tensor.matmul(
                                pss,
                                lhsT=kpt_b[:, lh, kt * 128 : (kt + 1) * 128],
                                rhs=qpt_b[:, h, ssl],
                                start=True,
                                stop=True,
                            )
                            et = epool.tile([P, 512], bf, tag="et", name=f"et{kt}")
                            nc.scalar.activation(
                                out=et[:], in_=pss[:], func=Exp, scale=scale
                            )
                            nc.tensor.matmul(
                                pso,
                                lhsT=vp_sb[:, kt, lh * 65 : (lh + 1) * 65],
                                rhs=et[:],
                                start=(kt == 0),
                                stop=(kt == 15),
                            )
                        recip = ntmp.tile([1, 512], f32, tag="recip")
                        nc.vector.reciprocal(recip[:], pso[64:65, :])
                        bc = ntmp.tile([64, 512], f32, tag="bc")
                        nc.gpsimd.partition_broadcast(bc[:], recip[:])
                        nc.vector.tensor_mul(
                            out=outT_b[hp, hc, ssl], in0=pso[0:64, :], in1=bc[:]
                        )

                # ---- output projection for this quarter's s-tiles ----
                for qi in range(4):
                    qt = quarter * 4 + qi
                    for dn in range(4):
                        psf = mpsum.tile([P, 512], f32, tag="pp", name="psf")
                        for cc in range(4):
                            nc.tensor.matmul(
                                psf,
                                lhsT=outT_b[:, cc, qt * 128 : (qt + 1) * 128],
                                rhs=wo_sb[:, cc, dn * 512 : (dn + 1) * 512],
                                start=(cc == 0),
                                stop=(cc == 3),
                            )
                        of = fout.tile([P, 512], f32, tag="of")
                        nc.any.tensor_copy(out=of[:], in_=psf[:])
                        nc.sync.dma_start(
                            out3[:, qt, dn * 512 : (dn + 1) * 512], of[:]
                        )

    nc.finalize()
    return nc


def _rot_cols(W):
    """(x @ rot_cols(W)) == rotate_half(x @ W), per 64-wide head block."""
    Wr = np.empty_like(W)
    n = W.shape[1] // HD
    for h in range(n):
        blk = W[:, h * HD : (h + 1) * HD]
        Wr[:, h * HD : h * HD + 32] = -blk[:, 32:64]
        Wr[:, h * HD + 32 : h * HD + 64] = blk[:, 0:32]
    return Wr


def _host_inputs(q, k, v, Wq, Wk, Wv, Wo):
    """Build the 8 per-core input dicts."""
    inv_freq = 1.0 / (THETA ** (np.arange(0, HD, 2, dtype=np.float32) / HD))
    t = np.arange(S, dtype=np.float32)
    freqs = np.einsum("i,j->ij", t, inv_freq)
    emb = np.concatenate([freqs, freqs], axis=-1)  # [S, 64]
    cosT = np.ascontiguousarray(np.cos(emb).T, dtype=np.float32)  # [64, S]
    sinT = np.ascontiguousarray(np.sin(emb).T, dtype=np.float32)
    cos_rep = np.concatenate([cosT, cosT], axis=0)  # [128, S]
    sin_rep = np.concatenate([sinT, sinT], axis=0)

    qT = [np.ascontiguousarray(q[b].T).astype(BF16) for b in range(B)]
    kTt = [np.ascontiguousarray(k[b].T).astype(BF16) for b in range(B)]
    vTt = [np.ascontiguousarray(v[b].T).astype(BF16) for b in range(B)]

    in_maps = []
    for c in range(NCORES):
        b, g = divmod(c, 4)
        qheads = [2 * g, 2 * g + 8, 2 * g + 16, 2 * g + 24,
                  2 * g + 1, 2 * g + 9, 2 * g + 17, 2 * g + 25]
        qcols = np.concatenate([np.arange(h * HD, (h + 1) * HD) for h in qheads])
        kvcols = np.arange(2 * g * HD, (2 * g + 2) * HD)

        wq_c = np.ascontiguousarray(Wq[:, qcols])
        wqt_np = np.concatenate([wq_c, _rot_cols(wq_c)], axis=1).astype(BF16)
        wk_c = np.ascontiguousarray(Wk[:, kvcols])
        wkt_np = np.concatenate([wk_c, _rot_cols(wk_c)], axis=1).astype(BF16)
        wv_np = np.ascontiguousarray(Wv[:, kvcols]).astype(BF16)
        wo_np = np.ascontiguousarray(Wo[qcols, :]).astype(BF16)

        in_maps.append({
            "qT": qT[b], "kT": kTt[b], "vT": vTt[b],
            "wqt": wqt_np, "wkt": wkt_np, "wv": wv_np, "wo": wo_np,
            "cosr": cos_rep, "sinr": sin_rep,
        })
    return in_maps


def kernel(q, k, v, attn_mask, Wq, Wk, Wv, Wo, bo):
    from concourse.bass_utils import run_bass_kernel_spmd

    q = np.asarray(q, dtype=np.float32)
    k = np.asarray(k, dtype=np.float32)
    v = np.asarray(v, dtype=np.float32)
    Wq = np.asarray(Wq, dtype=np.float32)
    Wk = np.asarray(Wk, dtype=np.float32)
    Wv = np.asarray(Wv, dtype=np.float32)
    Wo = np.asarray(Wo, dtype=np.float32)
    bo = np.asarray(bo, dtype=np.float32)

    if "nc" not in _CACHE:
        _CACHE["nc"] = _build_program()
    nc = _CACHE["nc"]

    in_maps = _host_inputs(q, k, v, Wq, Wk, Wv, Wo)
    trace = bool(int(os.environ.get("KERNEL_TRACE", "0")))
    tmpdir = os.environ.get("KERNEL_TRACE_DIR") or None
    res = run_bass_kernel_spmd(nc, in_maps, core_ids=list(range(NCORES)),
                               trace=trace, tmpdir=tmpdir)
    _CACHE["last_result"] = res

    out = np.zeros((B, S, D), dtype=np.float32)
    for c in range(NCORES):
        b = c // 4
        out[b] += np.asarray(res.results[c]["out"], dtype=np.float32)
    out += bo[None, None, :]
    return out



# revision 12
# speedup vs baseline: 1.5225x; 1.5225x over previous
"""GQA attention block on 8 trn2 NeuronCores.

Sharding: core c = (batch b=c//4, kv-head-pair g=c%4). Each core owns kv heads
{2g, 2g+1} and their 8 query heads (GQA tile mapping: q-head i -> kv-head i%8),
with Wq/Wk/Wv column-sharded and Wo row-sharded; host sums the 4 partial
outputs per batch (bf16 partials, fp32 sum) and adds bo.

Device strategy (per core):
  - host stages q^T/k^T/v^T (bf16) so every matmul has its contraction dim on
    partitions with no device-side transposes.
  - RoPE applied on DVE (partition-shifted rotate_half + cos/sin combine) for
    both Q and K; no doubled projection weights.
  - q heads are interleaved as (kv0-head j, kv1-head j) pairs so each score
    matmul pair runs ROW-TILED on the PE array (64x128 tiles T0/T8, concurrent)
    writing a 2-bank PSUM blob; one Exp ACT over the [128,1024] blob (scale=1/8
    folded, no max subtraction -- scores bounded ~|6|).
  - AV via lhsT=Vp with an appended ones column giving the softmax denominator
    for free; normalization via fast-approx reciprocal + partition broadcast.
  - out^T feeds the final projection as lhsT directly; partial [S,D] bf16 out.
"""

import os
from contextlib import ExitStack

import numpy as np
import ml_dtypes

D = 2048
QH = 32
KVH = 8
HD = 64
B = 2
S = 2048
THETA = 1000000.0
P = 128
NCORES = 8

BF16 = ml_dtypes.bfloat16

_CACHE = {}


def _build_program():
    import concourse.bass as bass
    import concourse.tile as tile
    from concourse import bacc, mybir

    nc = bacc.Bacc(
        "TRN2",
        target_bir_lowering=False,
        debug=False,
        enable_asserts=False,
        num_devices=NCORES,
    )
    bf = mybir.dt.bfloat16
    f32 = mybir.dt.float32
    Exp = mybir.ActivationFunctionType.Exp
    scale = 1.0 / float(np.sqrt(HD))

    qT = nc.dram_tensor("qT", [D, S], bf, kind="ExternalInput").ap()
    kT = nc.dram_tensor("kT", [D, S], bf, kind="ExternalInput").ap()
    vT = nc.dram_tensor("vT", [D, S], bf, kind="ExternalInput").ap()
    wq = nc.dram_tensor("wq", [D, 512], bf, kind="ExternalInput").ap()
    wk = nc.dram_tensor("wk", [D, 128], bf, kind="ExternalInput").ap()
    wv = nc.dram_tensor("wv", [D, 128], bf, kind="ExternalInput").ap()
    wo = nc.dram_tensor("wo", [512, D], bf, kind="ExternalInput").ap()
    cosr = nc.dram_tensor("cosr", [P, S], f32, kind="ExternalInput").ap()
    sinr = nc.dram_tensor("sinr", [P, S], f32, kind="ExternalInput").ap()
    out = nc.dram_tensor("out", [S, D], bf, kind="ExternalOutput").ap()

    # partitioned DRAM views
    qT3 = qT.rearrange("(o p) s -> p o s", p=P)    # [128, 16, 2048]
    kT3 = kT.rearrange("(o p) s -> p o s", p=P)
    vT3 = vT.rearrange("(o p) s -> p o s", p=P)
    wq3 = wq.rearrange("(o p) m -> p o m", p=P)    # [128, 16, 512]
    wk3 = wk.rearrange("(o p) m -> p o m", p=P)    # [128, 16, 128]
    wv3 = wv.rearrange("(o p) m -> p o m", p=P)    # [128, 16, 128]
    wo3 = wo.rearrange("(o p) d -> p o d", p=P)    # [128, 4, 2048]
    out3 = out.rearrange("(t p) d -> p t d", p=P)  # [128, 16, 2048]

    with tile.TileContext(nc) as tc, ExitStack() as ctx:
        const = ctx.enter_context(tc.tile_pool(name="const", bufs=1))
        persist = ctx.enter_context(tc.tile_pool(name="persist", bufs=1))

        # ---- resident weights / tables ----
        wq_sb = const.tile([P, 16, 512], bf, tag="wq")
        nc.sync.dma_start(wq_sb[:], wq3[:])
        wk_sb = const.tile([P, 16, 128], bf, tag="wk")
        nc.sync.dma_start(wk_sb[:], wk3[:])
        wv_sb = const.tile([P, 16, 128], bf, tag="wv")
        nc.sync.dma_start(wv_sb[:], wv3[:])
        wo_sb = const.tile([P, 4, 2048], bf, tag="wo")
        nc.sync.dma_start(wo_sb[:], wo3[:])
        cos_sb = const.tile([P, S], f32, tag="cos")
        nc.sync.dma_start(cos_sb[:], cosr[:])
        sin_sb = const.tile([P, S], f32, tag="sin")
        nc.sync.dma_start(sin_sb[:], sinr[:])

        # ---- persistent intermediates ----
        kpt_b = persist.tile([P, S], bf, tag="kpt")          # rotated K^T, kv0|kv1
        qpt_b = persist.tile([P, 4, S], bf, tag="qpt")       # rotated Q^T pairs
        vp_sb = persist.tile([P, 16, 130], bf, tag="vp")     # Vp + ones cols
        outT_b = persist.tile([P, 4, S], bf, tag="outT")     # normalized out^T
        nc.vector.memset(vp_sb[:, :, 64:65], 1.0)
        nc.vector.memset(vp_sb[:, :, 129:130], 1.0)

        def rope_combine(dst, ps, rot_pool, sl):
            """dst[128, 512] (bf16) = ps*cos + rotate_half(ps)*sin over slice sl."""
            rot = rot_pool.tile([P, 512], f32, tag="rot")
            for hh in range(2):
                b0 = hh * 64
                nc.vector.tensor_scalar_mul(
                    rot[b0 : b0 + 32, :], ps[b0 + 32 : b0 + 64, :], -1.0
                )
                nc.vector.tensor_copy(
                    out=rot[b0 + 32 : b0 + 64, :], in_=ps[b0 : b0 + 32, :]
                )
            t1 = rot_pool.tile([P, 512], f32, tag="t1")
            t2 = rot_pool.tile([P, 512], f32, tag="t2")
            nc.vector.tensor_mul(out=t1[:], in0=ps[:], in1=cos_sb[:, sl])
            nc.vector.tensor_mul(out=t2[:], in0=rot[:], in1=sin_sb[:, sl])
            nc.vector.tensor_add(out=dst, in0=t1[:], in1=t2[:])

        # =============== Phase 1: V + K projections ===============
        with ExitStack() as pctx:
            bigin = pctx.enter_context(tc.tile_pool(name="bigin", bufs=1))
            kstream = pctx.enter_context(tc.tile_pool(name="kstream", bufs=4))
            ptmp = pctx.enter_context(tc.tile_pool(name="ptmp", bufs=2))
            ppsum = pctx.enter_context(
                tc.tile_pool(name="ppsum", bufs=4, space="PSUM")
            )

            # ---- V projection: direct Vp [s,128] via lhsT = vT slices ----
            for quarter in range(4):
                vh_sb = bigin.tile([P, 16, 512], bf, tag="bigin")
                for o in range(16):
                    nc.sync.dma_start(
                        vh_sb[:, o, :], vT3[:, o, quarter * 512 : (quarter + 1) * 512]
                    )
                for st in range(4):  # s-tiles of 128 within this quarter
                    psv_full = ppsum.tile([P, 512], f32, tag="ppv", name="psv")
                    psv = psv_full[:, :128]
                    for o in range(16):
                        nc.tensor.matmul(
                            psv,
                            lhsT=vh_sb[:, o, st * 128 : (st + 1) * 128],
                            rhs=wv_sb[:, o, :],
                            start=(o == 0),
                            stop=(o == 15),
                        )
                    kt_idx = quarter * 4 + st
                    nc.vector.tensor_copy(out=vp_sb[:, kt_idx, 0:64], in_=psv[:, 0:64])
                    nc.vector.tensor_copy(
                        out=vp_sb[:, kt_idx, 65:129], in_=psv[:, 64:128]
                    )

            # ---- K projection + RoPE on DVE ----
            for ns in range(4):
                ps_k = ppsum.tile([P, 512], f32, tag="ppk")
                for o in range(16):
                    ktile = kstream.tile([P, 512], bf, tag="kt")
                    nc.sync.dma_start(
                        ktile[:], kT3[:, o, ns * 512 : (ns + 1) * 512]
                    )
                    nc.tensor.matmul(
                        ps_k,
                        lhsT=wk_sb[:, o, :],
                        rhs=ktile[:],
                        start=(o == 0),
                        stop=(o == 15),
                    )
                sl = slice(ns * 512, (ns + 1) * 512)
                rope_combine(kpt_b[:, sl], ps_k, ptmp, sl)

        # ======= per s-quarter: attention with interleaved Q/O proj =======
        with ExitStack() as mctx:
            bigin = mctx.enter_context(tc.tile_pool(name="bigin2", bufs=2))
            ptmp = mctx.enter_context(tc.tile_pool(name="ptmp2", bufs=2))
            qpool = mctx.enter_context(
                tc.tile_pool(name="qpool", bufs=1, space="PSUM")
            )
            opool = mctx.enter_context(
                tc.tile_pool(name="opool", bufs=1, space="PSUM")
            )
            spsum = mctx.enter_context(
                tc.tile_pool(name="spsum", bufs=2, space="PSUM")
            )
            avpsum = mctx.enter_context(
                tc.tile_pool(name="avpsum", bufs=1, space="PSUM")
            )
            epool = mctx.enter_context(tc.tile_pool(name="et", bufs=3))
            ntmp = mctx.enter_context(tc.tile_pool(name="ntmp", bufs=3))
            fout = mctx.enter_context(tc.tile_pool(name="fout", bufs=3))

            def qproj_dma(quarter):
                qh_sb = bigin.tile([P, 16, 512], bf, tag="bigin")
                for o in range(16):
                    nc.sync.dma_start(
                        qh_sb[:, o, :], qT3[:, o, quarter * 512 : (quarter + 1) * 512]
                    )
                return qh_sb

            def qproj_steps(quarter, qh_sb, pools):
                """64 tensor-step closures; each emits one matmul (rope attached
                to the last o of each m)."""
                gs = slice(quarter * 512, (quarter + 1) * 512)
                state = {}

                def step(m, o):
                    if o == 0:
                        pool, tag = pools[m % len(pools)]
                        state["ps"] = pool.tile(
                            [P, 512], f32, tag=tag, name=f"qp{quarter}_{m}"
                        )
                    nc.tensor.matmul(
                        state["ps"],
                        lhsT=wq_sb[:, o, m * 128 : (m + 1) * 128],
                        rhs=qh_sb[:, o, :],
                        start=(o == 0),
                        stop=(o == 15),
                    )
                    if o == 15:
                        rope_combine(qpt_b[:, m, gs], state["ps"], ptmp, gs)

                return [
                    (lambda m=m, o=o: step(m, o))
                    for m in range(4)
                    for o in range(16)
                ]

            def oproj_steps(quarter, pools):
                """64 tensor-step closures; copy+DMA attached to cc==3."""
                state = {}

                def step(i, qt, dn, cc):
                    if cc == 0:
                        pool, tag = pools[i % len(pools)]
                        state["psf"] = pool.tile(
                            [P, 512], f32, tag=tag, name=f"psf{quarter}_{i}"
                        )
                    nc.tensor.matmul(
                        state["psf"],
                        lhsT=outT_b[:, cc, qt * 128 : (qt + 1) * 128],
                        rhs=wo_sb[:, cc, dn * 512 : (dn + 1) * 512],
                        start=(cc == 0),
                        stop=(cc == 3),
                    )
                    if cc == 3:
                        of = fout.tile([P, 512], bf, tag="of")
                        nc.vector.tensor_copy(out=of[:], in_=state["psf"][:])
                        nc.sync.dma_start(
                            out3[:, qt, dn * 512 : (dn + 1) * 512], of[:]
                        )

                return [
                    (
                        lambda i=qi * 4 + dn, qt=quarter * 4 + qi, dn=dn, cc=cc:
                        step(i, qt, dn, cc)
                    )
                    for qi in range(4)
                    for dn in range(4)
                    for cc in range(4)
                ]

            def attention(quarter, feeds):
                gs = slice(quarter * 512, (quarter + 1) * 512)
                fed = 0
                nslots = 64
                for j in range(4):
                    avA = avpsum.tile([65, 512], f32, tag="avA")
                    avB = avpsum.tile([65, 512], f32, tag="avB")
                    for c in range(16):
                        cs = slice(c * 128, (c + 1) * 128)
                        sb = spsum.tile([P, 1024], f32, tag="sb")
                        nc.tensor.matmul(
                            sb[:, 0:512],
                            lhsT=kpt_b[0:64, cs],
                            rhs=qpt_b[0:64, j, gs],
                            start=True,
                            stop=True,
                            tile_position=(0, 0),
                        )
                        nc.tensor.matmul(
                            sb[:, 512:1024],
                            lhsT=kpt_b[64:128, cs],
                            rhs=qpt_b[64:128, j, gs],
                            start=True,
                            stop=True,
                            tile_position=(64, 0),
                        )
                        et = epool.tile([P, 1024], bf, tag="et", name=f"et{c}")
                        nc.scalar.activation(
                            out=et[:], in_=sb[:], func=Exp, scale=scale
                        )
                        nc.tensor.matmul(
                            avA[:],
                            lhsT=vp_sb[:, c, 0:65],
                            rhs=et[:, 0:512],
                            start=(c == 0),
                            stop=(c == 15),
                        )
                        nc.tensor.matmul(
                            avB[:],
                            lhsT=vp_sb[:, c, 65:130],
                            rhs=et[:, 512:1024],
                            start=(c == 0),
                            stop=(c == 15),
                        )
                        slot = j * 16 + c
                        want = (slot + 1) * len(feeds) // nslots
                        while fed < want:
                            feeds[fed]()
                            fed += 1
                    # custom-DVE ops need partition-0-based APs: stage the
                    # denominator rows at partition 0 first
                    denA = ntmp.tile([1, 512], f32, tag="denA")
                    denB = ntmp.tile([1, 512], f32, tag="denB")
                    nc.vector.tensor_copy(out=denA[:], in_=avA[64:65, :])
                    nc.vector.tensor_copy(out=denB[:], in_=avB[64:65, :])
                    recA = ntmp.tile([1, 512], f32, tag="recA")
                    recB = ntmp.tile([1, 512], f32, tag="recB")
                    nc.vector.reciprocal_approx_fast(out=recA[:], in_=denA[:])
                    nc.vector.reciprocal_approx_fast(out=recB[:], in_=denB[:])
                    bcA = ntmp.tile([64, 512], f32, tag="bcA")
                    bcB = ntmp.tile([64, 512], f32, tag="bcB")
                    nc.gpsimd.partition_broadcast(bcA[:], recA[:])
                    nc.gpsimd.partition_broadcast(bcB[:], recB[:])
                    nc.vector.tensor_mul(
                        out=outT_b[0:64, j, gs], in0=avA[0:64, :], in1=bcA[:]
                    )
                    nc.vector.tensor_mul(
                        out=outT_b[64:128, j, gs], in0=avB[0:64, :], in1=bcB[:]
                    )
                while fed < len(feeds):
                    feeds[fed]()
                    fed += 1

            def interleave(a, b):
                out = []
                for x, y in zip(a, b):
                    out.append(x)
                    out.append(y)
                out += a[len(b):] or b[len(a):]
                return out

            QP = (qpool, "qp")
            OP = (opool, "psf")

            # quarter 0's Q projection runs standalone up front
            qh0 = qproj_dma(0)
            for s in qproj_steps(0, qh0, [QP, OP]):
                s()
            qh_next = qproj_dma(1)
            for quarter in range(4):
                if quarter == 0:
                    feeds = qproj_steps(1, qh_next, [QP])
                elif quarter < 3:
                    feeds = interleave(
                        oproj_steps(quarter - 1, [OP]),
                        qproj_steps(quarter + 1, qh_next, [QP]),
                    )
                else:
                    feeds = oproj_steps(2, [OP])
                attention(quarter, feeds)
                if quarter < 2:
                    qh_next = qproj_dma(quarter + 2)
            # tail: final quarter's output projection (qpool free by now)
            for s in oproj_steps(3, [OP, QP]):
                s()

    nc.finalize()
    return nc


def _host_inputs(q, k, v, Wq, Wk, Wv, Wo):
    """Build the 8 per-core input dicts."""
    inv_freq = 1.0 / (THETA ** (np.arange(0, HD, 2, dtype=np.float32) / HD))
    t = np.arange(S, dtype=np.float32)
    freqs = np.einsum("i,j->ij", t, inv_freq)
    emb = np.concatenate([freqs, freqs], axis=-1)  # [S, 64]
    cosT = np.ascontiguousarray(np.cos(emb).T, dtype=np.float32)  # [64, S]
    sinT = np.ascontiguousarray(np.sin(emb).T, dtype=np.float32)
    cos_rep = np.concatenate([cosT, cosT], axis=0)  # [128, S]
    sin_rep = np.concatenate([sinT, sinT], axis=0)

    qT = [np.ascontiguousarray(q[b].T).astype(BF16) for b in range(B)]
    kTt = [np.ascontiguousarray(k[b].T).astype(BF16) for b in range(B)]
    vTt = [np.ascontiguousarray(v[b].T).astype(BF16) for b in range(B)]

    in_maps = []
    for c in range(NCORES):
        b, g = divmod(c, 4)
        # pair order: (kv0-head j, kv1-head j) interleaved
        qheads = [2 * g, 2 * g + 1, 2 * g + 8, 2 * g + 9,
                  2 * g + 16, 2 * g + 17, 2 * g + 24, 2 * g + 25]
        qcols = np.concatenate([np.arange(h * HD, (h + 1) * HD) for h in qheads])
        kvcols = np.arange(2 * g * HD, (2 * g + 2) * HD)

        wq_np = np.ascontiguousarray(Wq[:, qcols]).astype(BF16)
        wk_np = np.ascontiguousarray(Wk[:, kvcols]).astype(BF16)
        wv_np = np.ascontiguousarray(Wv[:, kvcols]).astype(BF16)
        wo_np = np.ascontiguousarray(Wo[qcols, :]).astype(BF16)

        in_maps.append({
            "qT": qT[b], "kT": kTt[b], "vT": vTt[b],
            "wq": wq_np, "wk": wk_np, "wv": wv_np, "wo": wo_np,
            "cosr": cos_rep, "sinr": sin_rep,
        })
    return in_maps


def kernel(q, k, v, attn_mask, Wq, Wk, Wv, Wo, bo):
    from concourse.bass_utils import run_bass_kernel_spmd

    q = np.asarray(q, dtype=np.float32)
    k = np.asarray(k, dtype=np.float32)
    v = np.asarray(v, dtype=np.float32)
    Wq = np.asarray(Wq, dtype=np.float32)
    Wk = np.asarray(Wk, dtype=np.float32)
    Wv = np.asarray(Wv, dtype=np.float32)
    Wo = np.asarray(Wo, dtype=np.float32)
    bo = np.asarray(bo, dtype=np.float32)

    if "nc" not in _CACHE:
        _CACHE["nc"] = _build_program()
    nc = _CACHE["nc"]

    in_maps = _host_inputs(q, k, v, Wq, Wk, Wv, Wo)
    trace = bool(int(os.environ.get("KERNEL_TRACE", "0")))
    tmpdir = os.environ.get("KERNEL_TRACE_DIR") or None
    res = run_bass_kernel_spmd(nc, in_maps, core_ids=list(range(NCORES)),
                               trace=trace, tmpdir=tmpdir)
    _CACHE["last_result"] = res

    out = np.zeros((B, S, D), dtype=np.float32)
    for c in range(NCORES):
        b = c // 4
        out[b] += np.asarray(res.results[c]["out"], dtype=np.float32)
    out += bo[None, None, :]
    return out


# revision 20
# speedup vs baseline: 1.6268x; 1.0685x over previous
"""GQA attention block on 8 trn2 NeuronCores.

Sharding: core c = (batch b=c//4, kv-head-pair g=c%4). Each core owns kv heads
{2g, 2g+1} and their 8 query heads (GQA tile mapping: q-head i -> kv-head i%8),
with Wq/Wk/Wv column-sharded and Wo row-sharded; host sums the 4 partial
outputs per batch (bf16 partials, fp32 sum) and adds bo.

Device strategy (per core):
  - host stages q^T/k^T/v^T (bf16) so every matmul has its contraction dim on
    partitions with no device-side transposes.
  - RoPE applied on DVE (partition-shifted rotate_half + cos/sin combine) for
    both Q and K; no doubled projection weights.
  - q heads are interleaved as (kv0-head j, kv1-head j) pairs so each score
    matmul pair runs ROW-TILED on the PE array (64x128 tiles T0/T8, concurrent)
    writing a 2-bank PSUM blob; one Exp ACT over the [128,1024] blob (scale=1/8
    folded, no max subtraction -- scores bounded ~|6|).
  - AV via lhsT=Vp with an appended ones column giving the softmax denominator
    for free; normalization via fast-approx reciprocal + partition broadcast.
  - out^T feeds the final projection as lhsT directly; partial [S,D] bf16 out.
"""

import os
from contextlib import ExitStack

import numpy as np
import ml_dtypes

D = 2048
QH = 32
KVH = 8
HD = 64
B = 2
S = 2048
THETA = 1000000.0
P = 128
NCORES = 8

BF16 = ml_dtypes.bfloat16

_CACHE = {}


def _build_program():
    import concourse.bass as bass
    import concourse.tile as tile
    from concourse import bacc, mybir

    nc = bacc.Bacc(
        "TRN2",
        target_bir_lowering=False,
        debug=False,
        enable_asserts=False,
        num_devices=NCORES,
    )
    bf = mybir.dt.bfloat16
    f32 = mybir.dt.float32
    Exp = mybir.ActivationFunctionType.Exp
    scale = 1.0 / float(np.sqrt(HD))

    qT = nc.dram_tensor("qT", [D, S], bf, kind="ExternalInput").ap()
    kT = nc.dram_tensor("kT", [D, S], bf, kind="ExternalInput").ap()
    vT = nc.dram_tensor("vT", [D, S], bf, kind="ExternalInput").ap()
    wq = nc.dram_tensor("wq", [D, 512], bf, kind="ExternalInput").ap()
    wk = nc.dram_tensor("wk", [D, 128], bf, kind="ExternalInput").ap()
    wv = nc.dram_tensor("wv", [D, 128], bf, kind="ExternalInput").ap()
    wo = nc.dram_tensor("wo", [512, D], bf, kind="ExternalInput").ap()
    cosr = nc.dram_tensor("cosr", [P, S], f32, kind="ExternalInput").ap()
    sinr = nc.dram_tensor("sinr", [P, S], f32, kind="ExternalInput").ap()
    out = nc.dram_tensor("out", [S, D], bf, kind="ExternalOutput").ap()

    # partitioned DRAM views
    qT3 = qT.rearrange("(o p) s -> p o s", p=P)    # [128, 16, 2048]
    kT3 = kT.rearrange("(o p) s -> p o s", p=P)
    vT3 = vT.rearrange("(o p) s -> p o s", p=P)
    wq3 = wq.rearrange("(o p) m -> p o m", p=P)    # [128, 16, 512]
    wk3 = wk.rearrange("(o p) m -> p o m", p=P)    # [128, 16, 128]
    wv3 = wv.rearrange("(o p) m -> p o m", p=P)    # [128, 16, 128]
    wo3 = wo.rearrange("(o p) d -> p o d", p=P)    # [128, 4, 2048]
    out3 = out.rearrange("(t p) d -> p t d", p=P)  # [128, 16, 2048]

    with tile.TileContext(nc) as tc, ExitStack() as ctx:
        const = ctx.enter_context(tc.tile_pool(name="const", bufs=1))
        persist = ctx.enter_context(tc.tile_pool(name="persist", bufs=1))

        # ---- resident weights / tables ----
        wq_sb = const.tile([P, 16, 512], bf, tag="wq")
        nc.sync.dma_start(wq_sb[:], wq3[:])
        wk_sb = const.tile([P, 16, 128], bf, tag="wk")
        nc.sync.dma_start(wk_sb[:], wk3[:])
        wv_sb = const.tile([P, 16, 128], bf, tag="wv")
        nc.sync.dma_start(wv_sb[:], wv3[:])
        wo_sb = const.tile([P, 4, 2048], bf, tag="wo")
        nc.sync.dma_start(wo_sb[:], wo3[:])
        cos_sb = const.tile([P, S], f32, tag="cos")
        nc.sync.dma_start(cos_sb[:], cosr[:])
        sin_sb = const.tile([P, S], f32, tag="sin")
        nc.sync.dma_start(sin_sb[:], sinr[:])

        # ---- persistent intermediates ----
        kpt_b = persist.tile([P, S], bf, tag="kpt")          # rotated K^T, kv0|kv1
        qpt_b = persist.tile([P, 4, S], bf, tag="qpt")       # rotated Q^T pairs
        vp_sb = persist.tile([P, 16, 130], bf, tag="vp")     # Vp + ones cols
        outT_b = persist.tile([P, 4, S], bf, tag="outT")     # normalized out^T
        nc.vector.memset(vp_sb[:, :, 64:65], 1.0)
        nc.vector.memset(vp_sb[:, :, 129:130], 1.0)

        def rope_combine(dst, ps, rot_pool, sl):
            """dst[128, 512] (bf16) = ps*cos + rotate_half(ps)*sin over slice sl."""
            rot = rot_pool.tile([P, 512], f32, tag="rot")
            for hh in range(2):
                b0 = hh * 64
                nc.vector.tensor_scalar_mul(
                    rot[b0 : b0 + 32, :], ps[b0 + 32 : b0 + 64, :], -1.0
                )
                nc.vector.tensor_copy(
                    out=rot[b0 + 32 : b0 + 64, :], in_=ps[b0 : b0 + 32, :]
                )
            t1 = rot_pool.tile([P, 512], f32, tag="t1")
            t2 = rot_pool.tile([P, 512], f32, tag="t2")
            nc.vector.tensor_mul(out=t1[:], in0=ps[:], in1=cos_sb[:, sl])
            nc.vector.tensor_mul(out=t2[:], in0=rot[:], in1=sin_sb[:, sl])
            nc.vector.tensor_add(out=dst, in0=t1[:], in1=t2[:])

        # ======= per s-quarter: attention with interleaved Q/O proj =======
        with ExitStack() as mctx:
            bigin = mctx.enter_context(tc.tile_pool(name="bigin2", bufs=2))
            ptmp = mctx.enter_context(tc.tile_pool(name="ptmp2", bufs=1))
            qpool = mctx.enter_context(
                tc.tile_pool(name="qpool", bufs=1, space="PSUM")
            )
            opool = mctx.enter_context(
                tc.tile_pool(name="opool", bufs=1, space="PSUM")
            )
            epool = mctx.enter_context(tc.tile_pool(name="et", bufs=3))
            ntmp = mctx.enter_context(tc.tile_pool(name="ntmp", bufs=1))
            fout = mctx.enter_context(tc.tile_pool(name="fout", bufs=3))

            def qproj_dma(quarter):
                qh_sb = bigin.tile([P, 16, 512], bf, tag="bigin")
                for o in range(16):
                    nc.sync.dma_start(
                        qh_sb[:, o, :], qT3[:, o, quarter * 512 : (quarter + 1) * 512]
                    )
                return qh_sb

            def qproj_steps(quarter, qh_sb, pools):
                """64 tensor-step closures; each emits one matmul (rope attached
                to the last o of each m)."""
                gs = slice(quarter * 512, (quarter + 1) * 512)
                state = {}

                def step(m, o):
                    if o == 0:
                        pool, tag = pools[m % len(pools)]
                        state["ps"] = pool.tile(
                            [P, 512], f32, tag=tag, name=f"qp{quarter}_{m}"
                        )
                    nc.tensor.matmul(
                        state["ps"],
                        lhsT=wq_sb[:, o, m * 128 : (m + 1) * 128],
                        rhs=qh_sb[:, o, :],
                        start=(o == 0),
                        stop=(o == 15),
                    )
                    if o == 15:
                        rope_combine(qpt_b[:, m, gs], state["ps"], ptmp, gs)

                return [
                    (lambda m=m, o=o: step(m, o))
                    for m in range(4)
                    for o in range(16)
                ]

            def oproj_steps(quarter, pools):
                """64 tensor-step closures; copy+DMA attached to cc==3."""
                state = {}

                def step(i, qt, dn, cc):
                    if cc == 0:
                        pool, tag = pools[i % len(pools)]
                        state["psf"] = pool.tile(
                            [P, 512], f32, tag=tag, name=f"psf{quarter}_{i}"
                        )
                    nc.tensor.matmul(
                        state["psf"],
                        lhsT=outT_b[:, cc, qt * 128 : (qt + 1) * 128],
                        rhs=wo_sb[:, cc, dn * 512 : (dn + 1) * 512],
                        start=(cc == 0),
                        stop=(cc == 3),
                    )
                    if cc == 3:
                        of = fout.tile([P, 512], bf, tag="of")
                        nc.vector.tensor_copy(out=of[:], in_=state["psf"][:])
                        nc.sync.dma_start(
                            out3[:, qt, dn * 512 : (dn + 1) * 512], of[:]
                        )

                return [
                    (
                        lambda i=qi * 4 + dn, qt=quarter * 4 + qi, dn=dn, cc=cc:
                        step(i, qt, dn, cc)
                    )
                    for qi in range(4)
                    for dn in range(4)
                    for cc in range(4)
                ]

            def attention(quarter, feeds):
                gs = slice(quarter * 512, (quarter + 1) * 512)
                fed = 0
                nslots = 64
                def emit_av(avA, avB, et, c):
                    nc.tensor.matmul(
                        avA[:],
                        lhsT=vp_sb[:, c, 0:65],
                        rhs=et[:, 0:512],
                        start=(c == 0),
                        stop=(c == 15),
                    )
                    nc.tensor.matmul(
                        avB[:],
                        lhsT=vp_sb[:, c, 65:130],
                        rhs=et[:, 512:1024],
                        start=(c == 0),
                        stop=(c == 15),
                    )

                for j in range(4):
                    avA = avpsum.tile([65, 512], f32, tag="avA")
                    avB = avpsum.tile([65, 512], f32, tag="avB")
                    pend = None  # (et, c) whose AV is deferred one slot
                    for c in range(16):
                        cs = slice(c * 128, (c + 1) * 128)
                        sb = spsum.tile([P, 1024], f32, tag="sb")
                        nc.tensor.matmul(
                            sb[:, 0:512],
                            lhsT=kpt_b[0:64, cs],
                            rhs=qpt_b[0:64, j, gs],
                            start=True,
                            stop=True,
                            tile_position=(0, 0),
                        )
                        nc.tensor.matmul(
                            sb[:, 512:1024],
                            lhsT=kpt_b[64:128, cs],
                            rhs=qpt_b[64:128, j, gs],
                            start=True,
                            stop=True,
                            tile_position=(64, 0),
                        )
                        et = epool.tile([P, 1024], bf, tag="et", name=f"et{c}")
                        nc.scalar.activation(
                            out=et[:], in_=sb[:], func=Exp, scale=scale
                        )
                        # feeds run while this chunk's exp is on the ACT engine
                        slot = j * 16 + c
                        want = (slot + 1) * len(feeds) // nslots
                        while fed < want:
                            feeds[fed]()
                            fed += 1
                        if pend is not None:
                            emit_av(avA, avB, *pend)
                        pend = (et, c)
                    emit_av(avA, avB, *pend)
                    # custom-DVE ops need partition-0-based APs: stage the
                    # denominator rows at partition 0 first
                    denA = ntmp.tile([1, 512], f32, tag="denA")
                    denB = ntmp.tile([1, 512], f32, tag="denB")
                    nc.vector.tensor_copy(out=denA[:], in_=avA[64:65, :])
                    nc.vector.tensor_copy(out=denB[:], in_=avB[64:65, :])
                    recA = ntmp.tile([1, 512], f32, tag="recA")
                    recB = ntmp.tile([1, 512], f32, tag="recB")
                    nc.vector.reciprocal_approx_fast(out=recA[:], in_=denA[:])
                    nc.vector.reciprocal_approx_fast(out=recB[:], in_=denB[:])
                    bcA = ntmp.tile([64, 512], f32, tag="bcA")
                    bcB = ntmp.tile([64, 512], f32, tag="bcB")
                    nc.gpsimd.partition_broadcast(bcA[:], recA[:])
                    nc.gpsimd.partition_broadcast(bcB[:], recB[:])
                    nc.vector.tensor_mul(
                        out=outT_b[0:64, j, gs], in0=avA[0:64, :], in1=bcA[:]
                    )
                    nc.vector.tensor_mul(
                        out=outT_b[64:128, j, gs], in0=avB[0:64, :], in1=bcB[:]
                    )
                while fed < len(feeds):
                    feeds[fed]()
                    fed += 1

            def interleave(a, b):
                out = []
                for x, y in zip(a, b):
                    out.append(x)
                    out.append(y)
                out += a[len(b):] or b[len(a):]
                return out

            QP = (qpool, "qp")
            OP = (opool, "psf")

            # ===== startup: V + K projections with qproj(0) interleaved =====
            qh0 = qproj_dma(0)
            q0_steps = qproj_steps(0, qh0, [QP, OP])
            q0_fed = 0
            with ExitStack() as pctx:
                kstream = pctx.enter_context(tc.tile_pool(name="kstream", bufs=8))
                vpsum = pctx.enter_context(
                    tc.tile_pool(name="vpsum", bufs=4, space="PSUM")
                )
                kpsum = pctx.enter_context(
                    tc.tile_pool(name="kpsum", bufs=2, space="PSUM")
                )

                for quarter in range(4):
                    # ---- V projection for this quarter: stream per-o tiles,
                    # one PSUM bank per s-tile accumulator ----
                    psv4 = [
                        vpsum.tile([P, 512], f32, tag="ppv", name=f"psv{st}")
                        for st in range(4)
                    ]
                    for o in range(16):
                        vtile = kstream.tile([P, 512], bf, tag="kt", name=f"vt{o}")
                        nc.sync.dma_start(
                            vtile[:], vT3[:, o, quarter * 512 : (quarter + 1) * 512]
                        )
                        for st in range(4):
                            nc.tensor.matmul(
                                psv4[st][:, 0:128],
                                lhsT=vtile[:, st * 128 : (st + 1) * 128],
                                rhs=wv_sb[:, o, :],
                                start=(o == 0),
                                stop=(o == 15),
                            )
                        if o % 4 == 3:
                            while q0_fed < (quarter * 16 + o + 1) * 64 // 80:
                                q0_steps[q0_fed]()
                                q0_fed += 1
                    for st in range(4):
                        kt_idx = quarter * 4 + st
                        nc.vector.tensor_copy(
                            out=vp_sb[:, kt_idx, 0:64], in_=psv4[st][:, 0:64]
                        )
                        nc.vector.tensor_copy(
                            out=vp_sb[:, kt_idx, 65:129], in_=psv4[st][:, 64:128]
                        )

                    # ---- K projection + RoPE for this quarter ----
                    ps_k = kpsum.tile([P, 512], f32, tag="ppk")
                    for o in range(16):
                        ktile = kstream.tile([P, 512], bf, tag="kt")
                        nc.sync.dma_start(
                            ktile[:], kT3[:, o, quarter * 512 : (quarter + 1) * 512]
                        )
                        nc.tensor.matmul(
                            ps_k,
                            lhsT=wk_sb[:, o, :],
                            rhs=ktile[:],
                            start=(o == 0),
                            stop=(o == 15),
                        )
                    sl = slice(quarter * 512, (quarter + 1) * 512)
                    rope_combine(kpt_b[:, sl], ps_k, ptmp, sl)
            while q0_fed < len(q0_steps):
                q0_steps[q0_fed]()
                q0_fed += 1

            # attention-phase PSUM pools (created after startup pools release)
            spsum = mctx.enter_context(
                tc.tile_pool(name="spsum", bufs=2, space="PSUM")
            )
            avpsum = mctx.enter_context(
                tc.tile_pool(name="avpsum", bufs=1, space="PSUM")
            )

            qh_next = qproj_dma(1)
            for quarter in range(4):
                if quarter == 0:
                    feeds = qproj_steps(1, qh_next, [QP])
                elif quarter < 3:
                    feeds = interleave(
                        oproj_steps(quarter - 1, [OP]),
                        qproj_steps(quarter + 1, qh_next, [QP]),
                    )
                else:
                    feeds = oproj_steps(2, [OP])
                attention(quarter, feeds)
                if quarter < 2:
                    qh_next = qproj_dma(quarter + 2)
            # tail: final quarter's output projection (qpool free by now)
            for s in oproj_steps(3, [OP, QP]):
                s()

    nc.finalize()
    return nc


def _host_inputs(q, k, v, Wq, Wk, Wv, Wo):
    """Build the 8 per-core input dicts."""
    inv_freq = 1.0 / (THETA ** (np.arange(0, HD, 2, dtype=np.float32) / HD))
    t = np.arange(S, dtype=np.float32)
    freqs = np.einsum("i,j->ij", t, inv_freq)
    emb = np.concatenate([freqs, freqs], axis=-1)  # [S, 64]
    cosT = np.ascontiguousarray(np.cos(emb).T, dtype=np.float32)  # [64, S]
    sinT = np.ascontiguousarray(np.sin(emb).T, dtype=np.float32)
    cos_rep = np.concatenate([cosT, cosT], axis=0)  # [128, S]
    sin_rep = np.concatenate([sinT, sinT], axis=0)

    qT = [np.ascontiguousarray(q[b].T).astype(BF16) for b in range(B)]
    kTt = [np.ascontiguousarray(k[b].T).astype(BF16) for b in range(B)]
    vTt = [np.ascontiguousarray(v[b].T).astype(BF16) for b in range(B)]

    in_maps = []
    for c in range(NCORES):
        b, g = divmod(c, 4)
        # pair order: (kv0-head j, kv1-head j) interleaved
        qheads = [2 * g, 2 * g + 1, 2 * g + 8, 2 * g + 9,
                  2 * g + 16, 2 * g + 17, 2 * g + 24, 2 * g + 25]
        qcols = np.concatenate([np.arange(h * HD, (h + 1) * HD) for h in qheads])
        kvcols = np.arange(2 * g * HD, (2 * g + 2) * HD)

        wq_np = np.ascontiguousarray(Wq[:, qcols]).astype(BF16)
        wk_np = np.ascontiguousarray(Wk[:, kvcols]).astype(BF16)
        wv_np = np.ascontiguousarray(Wv[:, kvcols]).astype(BF16)
        wo_np = np.ascontiguousarray(Wo[qcols, :]).astype(BF16)

        in_maps.append({
            "qT": qT[b], "kT": kTt[b], "vT": vTt[b],
            "wq": wq_np, "wk": wk_np, "wv": wv_np, "wo": wo_np,
            "cosr": cos_rep, "sinr": sin_rep,
        })
    return in_maps


def kernel(q, k, v, attn_mask, Wq, Wk, Wv, Wo, bo):
    from concourse.bass_utils import run_bass_kernel_spmd

    q = np.asarray(q, dtype=np.float32)
    k = np.asarray(k, dtype=np.float32)
    v = np.asarray(v, dtype=np.float32)
    Wq = np.asarray(Wq, dtype=np.float32)
    Wk = np.asarray(Wk, dtype=np.float32)
    Wv = np.asarray(Wv, dtype=np.float32)
    Wo = np.asarray(Wo, dtype=np.float32)
    bo = np.asarray(bo, dtype=np.float32)

    if "nc" not in _CACHE:
        _CACHE["nc"] = _build_program()
    nc = _CACHE["nc"]

    in_maps = _host_inputs(q, k, v, Wq, Wk, Wv, Wo)
    trace = bool(int(os.environ.get("KERNEL_TRACE", "0")))
    tmpdir = os.environ.get("KERNEL_TRACE_DIR") or None
    res = run_bass_kernel_spmd(nc, in_maps, core_ids=list(range(NCORES)),
                               trace=trace, tmpdir=tmpdir)
    _CACHE["last_result"] = res

    out = np.zeros((B, S, D), dtype=np.float32)
    for c in range(NCORES):
        b = c // 4
        out[b] += np.asarray(res.results[c]["out"], dtype=np.float32)
    out += bo[None, None, :]
    return out
